# revision 20
# baseline (speedup 1.0000x reference)
"""Trainium2 Bass kernel for DiffusionHungarianMatcher (DETR-style matcher).

reference semantics:
  C[b]   = cost_class + 5*L1(cdist) - 2*GIoU          [B, Q=900, T=128]
  greedy = 128 sequential global-argmin picks with row/col elimination
  returns (src_idx [B,T] i32, tgt_idx [B,T] i32, C [B,Q,T] f32)

Device strategy (8 NeuronCores, data-parallel over batch, 8 images/core):
  - compute G = -C^T in [T=128 partitions, Q free] layout per image:
      * softmax-prob gather via one-hot fp32 matmul on PE (exact)
      * box terms via fused custom DVE ops (2 streams + per-partition scalars)
      * q-broadcast rows via GPSIMD partition_broadcast
  - extract per-row segmented top-8 candidate caches (4 segs x top-8 = 32)
    with vector.max / vector.max_index
  - DMA out G (=-C^T) and the caches.
Host: shard/pad inputs, run SPMD on cores 0-7, then run the exact greedy scan
from the per-row caches (provably sufficient per simulation; has an exact
repair path reading G rows if a cache segment ever exhausts).
"""

import numpy as np

B, Q, T, NCLS = 64, 900, 128, 91
QPAD = 1024
N_IMG = 8  # images per core
N_CORES = 8
NSEG, TOPK = 4, 8
SEG = Q // NSEG  # 225
K = NSEG * TOPK  # 32
COST_CLASS, COST_BBOX, COST_GIOU = 1.0, 5.0, 2.0
PAD_LOGIT = 0.0  # pad rows are never read downstream; 0 keeps all math finite
PAD_BOX = 0.5

_NC_CACHE = {}
_OPS_CACHE = {}

# test-harness hooks: set TRACE=True before calling kernel() to capture an
# NTFF profile; the BassKernelResults lands in LAST_RESULT.
TRACE = False
LAST_RESULT = None


# --------------------------------------------------------------------------- #
# custom DVE ops
# --------------------------------------------------------------------------- #
def _get_dve_ops():
    if _OPS_CACHE:
        return _OPS_CACHE
    from concourse.dve_ops import DveOp, OPS
    from concourse.dve_spec import (
        Spec, Src0, Src1, C0, C1, C2, Zero, relu, maxx, minn, Bin, AluOp, lower,
    )
    from concourse.dve_uop import DveOpSpec
    from concourse.dve_ops import get_dve_sub_opcode
    from concourse.dve_spec import spec_leaves

    def absd(a, b):
        return Bin(AluOp.ABSOLUTE_DIFF, a, b)

    # inputs: Src0 = pc_b (center coord, broadcast over partitions),
    #         Src1 = ps_b (size coord), C0 = t_lo[p], C1 = t_hi[p], imm2 = 0.5
    _h = Src1 * C2
    specs = {
        # relu(min(p_hi, t_hi) - max(p_lo, t_lo)) — 1D overlap extent
        "HM_OVLP": Spec(
            body=relu(minn(Src0 + _h, C1) - maxx(Src0 - _h, C0)),
            reference=lambda in0, in1, s0, s1, imm2: np.maximum(
                np.minimum(in0 + in1 * imm2, s1) - np.maximum(in0 - in1 * imm2, s0),
                0.0,
            ).astype(np.float32),
        ),
        # max(p_hi, t_hi) - min(p_lo, t_lo) — 1D enclosing extent
        "HM_ENCL": Spec(
            body=maxx(Src0 + _h, C1) - minn(Src0 - _h, C0),
            reference=lambda in0, in1, s0, s1, imm2: (
                np.maximum(in0 + in1 * imm2, s1) - np.minimum(in0 - in1 * imm2, s0)
            ).astype(np.float32),
        ),
        # |pc - tc| + |ps - ts|   (C0 = t_center, C1 = t_size)
        "HM_L1P": Spec(
            body=absd(Src0, C0) + absd(Src1, C1),
            reference=lambda in0, in1, s0, s1, imm2: (
                np.abs(in0 - s0) + np.abs(in1 - s1)
            ).astype(np.float32),
        ),
    }

    ops = {}
    existing = {op.name for op in OPS}
    for name, spec in specs.items():
        if name in existing:
            for op in OPS:
                if op.name == name:
                    ops[name] = op
            continue
        op = DveOp(name, spec, subdim=False, uops_sha={})
        OPS.append(op)
        # register in the module-level lookup tables built at import time
        import concourse.dve_ops as dvo
        dvo._SUB_OPCODE_FOR_NAME[name] = dvo._CUSTOM_DVE_ROW_BASE + len(OPS) - 1
        dvo.CUSTOM_DVE_SPECS[name] = spec
        assert max(dvo._SUB_OPCODE_FOR_NAME.values()) < 0x20
        # pin the shas (computed locally; validated by DveOp.compile)
        from concourse.dve_ops import has_src1
        for ver in ("v3", "v4"):
            s = DveOpSpec(
                name=name,
                opcode=get_dve_sub_opcode(name),
                uops=lower(spec, ver=ver),
                rd1_en=has_src1(spec),
            )
            op.uops_sha[ver] = s.sha(ver)
        ops[name] = op
    _OPS_CACHE.update(ops)
    return _OPS_CACHE


# --------------------------------------------------------------------------- #
# device program
# --------------------------------------------------------------------------- #
def build_nc(n_img=N_IMG, n_cores=N_CORES):
    import concourse.bass as bass
    import concourse.mybir as mybir
    from concourse import bacc
    from concourse.tile import TileContext

    ops = _get_dve_ops()
    OV, EN, L1P = ops["HM_OVLP"], ops["HM_ENCL"], ops["HM_L1P"]

    f32 = mybir.dt.float32
    u32 = mybir.dt.uint32
    Alu = mybir.AluOpType

    # Bacc (not plain Bass): Tile emits raw ISA (EVSEM barrier etc.) that
    # needs Bacc's lowering before walrus codegen. debug=False for axon.
    nc = bacc.Bacc("TRN2", target_bir_lowering=False, debug=False,
                   num_devices=n_cores)
    lg_p = nc.declare_dram_parameter("lg", [n_img, QPAD, NCLS], f32, isOutput=False)
    pb_p = nc.declare_dram_parameter("pb", [n_img, QPAD, 4], f32, isOutput=False)
    tb_p = nc.declare_dram_parameter("tb", [n_img, T, 4], f32, isOutput=False)
    lab_p = nc.declare_dram_parameter("lab", [n_img, 1, T], f32, isOutput=False)
    cst_p = nc.declare_dram_parameter("cst", [128, 129], f32, isOutput=False)
    ct_p = nc.declare_dram_parameter("ct", [n_img, T, Q], f32, isOutput=True)
    cv_p = nc.declare_dram_parameter("cachev", [n_img, T, K], f32, isOutput=True)
    ci_p = nc.declare_dram_parameter("cachei", [n_img, T, K], u32, isOutput=True)
    # DRAM scratch used to collapse the 32-partition pb-transpose into 4 rows
    pbs_p = nc.dram_tensor("pbt_scratch", [n_img, 32, 128], f32)

    with TileContext(nc) as tc:
        from contextlib import ExitStack
        with ExitStack() as ctx:
            const = ctx.enter_context(tc.tile_pool(name="const", bufs=1))
            p_lg = ctx.enter_context(tc.tile_pool(name="lg", bufs=2))
            p_big = ctx.enter_context(tc.tile_pool(name="big", bufs=1))
            p_g = ctx.enter_context(tc.tile_pool(name="g", bufs=2))
            p_sm = ctx.enter_context(tc.tile_pool(name="sm", bufs=4))
            p_oh = ctx.enter_context(tc.tile_pool(name="oh", bufs=2))
            p_cache = ctx.enter_context(tc.tile_pool(name="cache", bufs=2))
            ps_tr = ctx.enter_context(tc.tile_pool(name="ps_tr", bufs=1, space="PSUM"))
            ps_eg = ctx.enter_context(tc.tile_pool(name="ps_eg", bufs=2, space="PSUM"))
            ps_sm = ctx.enter_context(tc.tile_pool(name="ps_sm", bufs=2, space="PSUM"))

            from concourse import library_config
            nc.gpsimd.load_library(library_config.attnmlp)

            cst_t = const.tile([128, 129], f32)
            nc.sync.dma_start(out=cst_t[:], in_=cst_p[:, :])
            iota91 = cst_t[:, 0:1]          # [:91] used
            ident = cst_t[:, 1:129]         # [128,128] identity

            for i in range(n_img):
                # ---------------- loads ----------------
                lg_t = p_lg.tile([128, 8, NCLS], f32, tag="lg")
                nc.sync.dma_start(
                    out=lg_t[:], in_=lg_p[i].rearrange("(k p) c -> p k c", p=128)
                )
                pb_t = p_sm.tile([128, 32], f32, tag="pb")
                nc.sync.dma_start(
                    out=pb_t.rearrange("p (k d) -> p k d", k=8),
                    in_=pb_p[i].rearrange("(k p) d -> p k d", p=128),
                )
                tb_t = p_sm.tile([128, 4], f32, tag="tb")
                nc.sync.dma_start(out=tb_t[:], in_=tb_p[i])
                lab_t = p_sm.tile([128, T], f32, tag="lab")
                nc.sync.dma_start(out=lab_t[:1, :], in_=lab_p[i])

                # ---------------- softmax probs ----------------
                e_t = p_lg.tile([128, 8, NCLS], f32, tag="e")
                nc.scalar.activation(
                    e_t[:], lg_t[:], mybir.ActivationFunctionType.Exp
                )
                rs_t = p_sm.tile([128, 8], f32, tag="rs")
                nc.vector.tensor_reduce(
                    rs_t[:], e_t[:], axis=mybir.AxisListType.X, op=Alu.add
                )
                rsr_t = p_sm.tile([128, 8], f32, tag="rsr")
                nc.vector.reciprocal(rsr_t[:], rs_t[:])
                for kchunk in range(8):
                    nc.vector.tensor_scalar_mul(
                        out=e_t[:, kchunk, :],
                        in0=e_t[:, kchunk, :],
                        scalar1=rsr_t[:, kchunk : kchunk + 1],
                    )

                # transpose probs: 8 x [128,91] -> psum [91, 1024]
                pt_ps = ps_tr.tile([128, 1024], f32, tag="ptps")
                for kchunk in range(8):
                    nc.tensor.transpose(
                        pt_ps[:NCLS, kchunk * 128 : (kchunk + 1) * 128],
                        e_t[:, kchunk, :],
                        ident,
                    )
                probT = p_big.tile([128, 1024], f32, tag="probT", bufs=2)
                nc.scalar.copy(out=probT[:NCLS, :], in_=pt_ps[:NCLS, :])

                # ---------------- one-hot & gather ----------------
                labrep = p_oh.tile([128, T], f32, tag="labrep")
                nc.gpsimd.partition_broadcast(labrep[:NCLS, :], lab_t[:1, :])
                onehot = p_oh.tile([128, T], f32, tag="onehot")
                nc.vector.tensor_scalar(
                    out=onehot[:NCLS, :],
                    in0=labrep[:NCLS, :],
                    scalar1=iota91[:NCLS, :],
                    scalar2=None,
                    op0=Alu.is_equal,
                )
                eg_ps = ps_eg.tile([128, Q], f32, tag="eg")
                nc.tensor.matmul(
                    eg_ps[:, 0:512], onehot[:NCLS, :], probT[:NCLS, 0:512]
                )
                nc.tensor.matmul(
                    eg_ps[:, 512:Q], onehot[:NCLS, :], probT[:NCLS, 512:Q]
                )

                # ---------------- pb transpose + broadcast rows ----------------
                pbr = p_sm.tile([128, 32], f32, tag="pbr")  # (d,k) order
                nc.vector.tensor_copy(
                    out=pbr.rearrange("p (d k) -> p d k", d=4),
                    in_=pb_t.rearrange("p (k d) -> p d k", d=4),
                )
                pbt_ps = ps_sm.tile([128, 128], f32, tag="pbtps")
                nc.tensor.transpose(pbt_ps[:32, :], pbr[:], ident)
                pbT = p_sm.tile([128, 128], f32, tag="pbT")
                nc.vector.tensor_copy(out=pbT[:32, :], in_=pbt_ps[:32, :])
                # collapse 32 partitions -> 4 rows of 1024 via DRAM bounce
                nc.sync.dma_start(out=pbs_p[i], in_=pbT[:32, :])
                rowsd = pbs_p[i].rearrange("(d k) p -> d (k p)", d=4)
                rows = []
                for d in range(4):
                    r_t = p_sm.tile([1, 1024], f32, tag=f"row{d}", name=f"row{d}_{i}")
                    nc.sync.dma_start(out=r_t[:], in_=rowsd[d : d + 1])
                    rows.append(r_t)

                bcx = p_big.tile([128, Q], f32, tag="bcx", bufs=2)
                bcy = p_big.tile([128, Q], f32, tag="bcy", bufs=2)
                bcw = p_big.tile([128, Q], f32, tag="bcw", bufs=2)
                bch = p_big.tile([128, Q], f32, tag="bch", bufs=2)
                nc.gpsimd.partition_broadcast(bcx[:], rows[0][:, :Q])
                nc.gpsimd.partition_broadcast(bcy[:], rows[1][:, :Q])
                nc.gpsimd.partition_broadcast(bcw[:], rows[2][:, :Q])
                nc.gpsimd.partition_broadcast(bch[:], rows[3][:, :Q])

                # ---------------- per-partition target scalars ----------------
                tcx, tcy = tb_t[:, 0:1], tb_t[:, 1:2]
                tw, th = tb_t[:, 2:3], tb_t[:, 3:4]
                a2 = p_sm.tile([128, 1], f32, tag="a2")
                nc.vector.tensor_mul(out=a2[:], in0=tw, in1=th)
                # t12: (tx1, ty1, tx2, ty2) per partition
                t12 = p_sm.tile([128, 4], f32, tag="t12")
                nc.vector.scalar_tensor_tensor(
                    out=t12[:, 0:2], in0=tb_t[:, 2:4], scalar=-0.5,
                    in1=tb_t[:, 0:2], op0=Alu.mult, op1=Alu.add,
                )
                nc.vector.scalar_tensor_tensor(
                    out=t12[:, 2:4], in0=tb_t[:, 2:4], scalar=0.5,
                    in1=tb_t[:, 0:2], op0=Alu.mult, op1=Alu.add,
                )

                # ---------------- geometry (custom DVE) ----------------
                def big(tag):
                    return p_big.tile([128, Q], f32, tag=tag, name=f"{tag}_{i}")

                area1 = big("area1")
                nc.vector.tensor_mul(out=area1[:], in0=bcw[:], in1=bch[:])
                tx1, ty1 = t12[:, 0:1], t12[:, 1:2]
                tx2, ty2 = t12[:, 2:3], t12[:, 3:4]
                wx, Wx, Lx = big("wx"), big("Wx"), big("Lx")
                nc.vector._custom_dve(OV, out=wx[:], in0=bcx[:], in1=bcw[:],
                                      s0=tx1, s1=tx2, imm2=0.5)
                nc.vector._custom_dve(EN, out=Wx[:], in0=bcx[:], in1=bcw[:],
                                      s0=tx1, s1=tx2, imm2=0.5)
                nc.vector._custom_dve(L1P, out=Lx[:], in0=bcx[:], in1=bcw[:],
                                      s0=tcx, s1=tw)
                wy, Wy, Ly = big("wy"), big("Wy"), big("Ly")
                nc.vector._custom_dve(OV, out=wy[:], in0=bcy[:], in1=bch[:],
                                      s0=ty1, s1=ty2, imm2=0.5)
                nc.vector._custom_dve(EN, out=Wy[:], in0=bcy[:], in1=bch[:],
                                      s0=ty1, s1=ty2, imm2=0.5)
                nc.vector._custom_dve(L1P, out=Ly[:], in0=bcy[:], in1=bch[:],
                                      s0=tcy, s1=th)

                inter = big("inter")
                nc.vector.tensor_mul(out=inter[:], in0=wx[:], in1=wy[:])
                areac = big("areac")
                nc.vector.tensor_mul(out=areac[:], in0=Wx[:], in1=Wy[:])
                union = big("union")
                nc.vector.scalar_tensor_tensor(
                    out=union[:], in0=area1[:], scalar=a2[:], in1=inter[:],
                    op0=Alu.add, op1=Alu.subtract,
                )
                scratch = big("scratch")
                ru = big("ru")
                nc.vector.reciprocal_approx_accurate(ru[:], union[:], scratch[:])
                rac = big("rac")
                nc.vector.reciprocal_approx_accurate(rac[:], areac[:], scratch[:])
                t1 = big("t1")
                nc.vector.tensor_mul(out=t1[:], in0=inter[:], in1=ru[:])
                t2 = big("t2")
                nc.vector.tensor_mul(out=t2[:], in0=union[:], in1=rac[:])
                s2 = big("s2")
                nc.vector.tensor_add(out=s2[:], in0=t1[:], in1=t2[:])
                s1 = big("s1")
                nc.vector.tensor_add(out=s1[:], in0=Lx[:], in1=Ly[:])
                pp = big("pp")
                nc.vector.tensor_scalar_add(out=pp[:], in0=eg_ps[:, :Q], scalar1=-2.0)
                s3 = big("s3")
                nc.vector.scalar_tensor_tensor(
                    out=s3[:], in0=s2[:], scalar=2.0, in1=pp[:],
                    op0=Alu.mult, op1=Alu.add,
                )
                g_t = p_g.tile([128, Q], f32, tag="G")
                nc.vector.scalar_tensor_tensor(
                    out=g_t[:], in0=s1[:], scalar=-5.0, in1=s3[:],
                    op0=Alu.mult, op1=Alu.add,
                )

                # ---------------- caches ----------------
                cv_t = p_cache.tile([128, K], f32, tag="cv")
                ci_t = p_cache.tile([128, K], u32, tag="ci")
                for s in range(NSEG):
                    seg = g_t[:, s * SEG : (s + 1) * SEG]
                    nc.vector.max(cv_t[:, s * TOPK : (s + 1) * TOPK], seg)
                    nc.vector.max_index(
                        ci_t[:, s * TOPK : (s + 1) * TOPK],
                        cv_t[:, s * TOPK : (s + 1) * TOPK],
                        seg,
                    )

                # ---------------- stores ----------------
                nc.sync.dma_start(out=ct_p[i], in_=g_t[:])
                nc.sync.dma_start(out=cv_p[i], in_=cv_t[:])
                nc.sync.dma_start(out=ci_p[i], in_=ci_t[:])

    nc.compile()  # Bacc register allocation / DCE / nop-fusion
    return nc


# --------------------------------------------------------------------------- #
# host-side exact greedy from caches (with exact repair path)
# --------------------------------------------------------------------------- #
def _greedy_from_cache(Gimg, cv, cq):
    """Exact greedy matching for one image.

    Gimg: [T, Q] (= -C^T), cv/cq: [T, K] segmented top-8 caches (desc per seg).
    Returns (src q-idx [T], tgt t-idx [T]) in greedy pick order.
    """
    Tn, K_ = cv.shape
    cv = cv.copy()
    cq = cq.astype(np.int64)
    # per (row, segment) count of still-valid cached entries
    seg_of_slot = np.repeat(np.arange(NSEG), TOPK)
    valid = np.ones((Tn, K_), bool)
    alive = np.ones(Tn, bool)
    usedq = np.zeros(Q, bool)
    suspect = np.zeros(Tn, bool)
    src, tgt = [], []
    NEG = -np.float32(np.inf)
    for _ in range(Tn):
        rowmax = cv.max(axis=1)
        rowmax[~alive] = NEG
        if (alive & suspect).any():
            # suspect rows' cache maxes may under-report; rescan them exactly
            best_v, t, q = NEG, -1, -1
            for t2 in np.where(alive & suspect)[0]:
                grow = np.where(usedq, NEG, Gimg[t2])
                q2 = int(np.argmax(grow))
                if grow[q2] > best_v:
                    best_v, t, q = grow[q2], int(t2), q2
            rowmax_ns = rowmax.copy()
            rowmax_ns[suspect] = NEG
            t3 = int(np.argmax(rowmax_ns))
            if alive[t3] and rowmax_ns[t3] > best_v:
                t = t3
                q = int(cq[t, int(np.argmax(cv[t]))])
        else:
            t = int(np.argmax(rowmax))
            q = int(cq[t, int(np.argmax(cv[t]))])
        src.append(q)
        tgt.append(t)
        usedq[q] = True
        alive[t] = False
        cv[t, :] = NEG
        # invalidate entries pointing at q
        hit = (cq == q) & valid
        if hit.any():
            cv[hit] = NEG
            valid[hit] = False
            # segment exhaustion -> mark suspect
            for (tt, ss) in zip(*np.where(hit)):
                segslots = slice(seg_of_slot[ss] * TOPK, (seg_of_slot[ss] + 1) * TOPK)
                if alive[tt] and not valid[tt, segslots].any():
                    suspect[tt] = True
    return np.array(src, np.int32), np.array(tgt, np.int32)


def _ensure_axon_ntff_hook():
    """Register the axon NTFF-profile hook that bass_utils expects; the image's
    antenv package lacks axon_hooks, so synthesize the module."""
    import sys
    import types
    try:
        from antenv.axon_hooks import get_axon_ntff_profile_hook  # noqa: F401
        return
    except ImportError:
        pass
    hook = None
    try:
        from trn_agent_boot.trn_boot import _ntff_profile_via_ctypes
        hook = _ntff_profile_via_ctypes("/opt/axon/libaxon_pjrt.so")
    except Exception:
        hook = None
    m = types.ModuleType("antenv.axon_hooks")
    state = {"hook": hook}
    m.get_axon_ntff_profile_hook = lambda: state["hook"]
    m.set_axon_ntff_profile_hook = lambda h: state.__setitem__("hook", h)
    sys.modules["antenv.axon_hooks"] = m


# --------------------------------------------------------------------------- #
# entry point
# --------------------------------------------------------------------------- #
def kernel(pred_logits, pred_boxes, tgt_labels, tgt_boxes):
    from concourse.bass_utils import run_bass_kernel_spmd

    if TRACE:
        _ensure_axon_ntff_hook()

    pred_logits = np.asarray(pred_logits, np.float32)
    pred_boxes = np.asarray(pred_boxes, np.float32)
    tgt_labels = np.asarray(tgt_labels)
    tgt_boxes = np.asarray(tgt_boxes, np.float32)

    if "nc" not in _NC_CACHE:
        _NC_CACHE["nc"] = build_nc(N_IMG)
    nc = _NC_CACHE["nc"]

    # pad inputs
    lg = np.full((B, QPAD, NCLS), PAD_LOGIT, np.float32)
    lg[:, :Q, :] = pred_logits
    pb = np.full((B, QPAD, 4), PAD_BOX, np.float32)
    pb[:, :Q, :] = pred_boxes
    lab = tgt_labels.astype(np.float32).reshape(B, 1, T)
    cst = np.zeros((128, 129), np.float32)
    cst[:, 0] = np.arange(128, dtype=np.float32)
    cst[:, 1:] = np.eye(128, dtype=np.float32)

    in_maps = []
    for c in range(N_CORES):
        sl = slice(c * N_IMG, (c + 1) * N_IMG)
        in_maps.append({
            "lg": np.ascontiguousarray(lg[sl]),
            "pb": np.ascontiguousarray(pb[sl]),
            "tb": np.ascontiguousarray(tgt_boxes[sl]),
            "lab": np.ascontiguousarray(lab[sl]),
            "cst": cst,
        })

    global LAST_RESULT
    res = run_bass_kernel_spmd(
        nc, in_maps, core_ids=list(range(N_CORES)), trace=TRACE
    )
    LAST_RESULT = res

    Gb = np.concatenate([r["ct"] for r in res.results], 0)          # [B, T, Q]
    cvb = np.concatenate([r["cachev"] for r in res.results], 0)      # [B, T, K]
    cib = np.concatenate([r["cachei"] for r in res.results], 0)      # [B, T, K]

    C = -np.ascontiguousarray(np.swapaxes(Gb, 1, 2))                 # [B, Q, T]

    seg_off = np.repeat(np.arange(NSEG) * SEG, TOPK)[None, None, :]
    cq = cib.astype(np.int64) + seg_off

    src = np.empty((B, T), np.int32)
    tgt = np.empty((B, T), np.int32)
    for b in range(B):
        s, t = _greedy_from_cache(Gb[b], cvb[b], cq[b])
        src[b], tgt[b] = s, t
    return src, tgt, C
